# revision 6
# baseline (speedup 1.0000x reference)
"""Trainium2 Bass kernel for the CGC (Customized Gate Control) MoE routing module.

Contract: kernel(**inputs) takes the FULL unsharded inputs (numpy/jax arrays)
and returns the FULL output [5, 16384, 256] float32.

Strategy:
  - Data-parallel over batch across 8 NeuronCores (2048 rows/core).
  - Host prep: per-core x slices fed pre-transposed [DIN, B_c] (contraction dim
    on SBUF partitions, fully contiguous DMAs, no on-device transposes);
    weights replicated, packed [DIN, E*H], cast to bf16 (fp32 matmul is a
    2-pass HI/LO op on the TRN2 PE - half throughput). PSUM stays fp32.
  - Expert biases enter PSUM via a K=32 one-hot matmul.
  - All gate logits for a group live in ONE PSUM bank (one bias matmul +
    region-wise accumulation); the gate matmuls of group g+1 are interleaved
    into group g's expert matmul stream so their LDWEIGHTS hide under the
    512-column expert matmuls and the PE never idles (HAM stays at 8/8).
  - Softmax runs batched over all domains x tiles of a group, with broadcast
    (step-0) tensor_tensor ops for the normalizations.
  - The gated combine uses a runtime-registered custom DVE op RELU_MAC:
        out = max(in0*s0 + in1, in1)  ==  s0*relu(in0) + in1   (s0 >= 0)
    reading expert PSUM banks directly - no relu evictions to SBUF. First
    terms are fused into ScalarE scaled-relu evictions; the out_sh
    shared-expert adds run on GpSimd to offload the Vector engine.
  - The double-softmax mask is known from sim_domain at trace time; masked
    shared-expert terms are not emitted (kernel is compile-specialized).
"""

import sys

sys.path.insert(0, "/opt/trn_rl_repo")

import numpy as np

D_NUM = 4
N_ES = 2
N_SH = 4
DIN = 512
H = 256
B = 16384
N_CORES = 8
BC = B // N_CORES          # 2048 rows per core
KC = DIN // 128            # 4 contraction chunks
GRP = 4                    # batch tiles (of 128 rows) per group
NG = BC // (128 * GRP)     # groups per core

# bias row layout: [spec d*512 | shared 1024 | (pad 36) | gate-bank]
OFF_SH = D_NUM * 512                     # 2048
OFF_GB = OFF_SH + N_SH * H + 36          # 3108 gate-bank bias region
GB_GS = D_NUM * GRP * 6                  # gsh region offset inside gate bank
NGB = GB_GS + GRP * 12
NB = OFF_GB + NGB

_BUILD_CACHE = {}
_RELU_MAC = None


def _get_relu_mac():
    """Register the RELU_MAC custom DVE op (idempotent)."""
    global _RELU_MAC
    if _RELU_MAC is not None:
        return _RELU_MAC
    from concourse import dve_ops
    from concourse.dve_spec import Spec, Src0, Src1, C0, maxx, lower, _has_src1
    from concourse.dve_uop import DveOpSpec

    name = "RELU_MAC_ANT"
    for o in dve_ops.OPS:
        if o.name == name:
            _RELU_MAC = o
            return o
    spec = Spec(
        body=maxx(Src0 * C0 + Src1, Src1),
        reference=lambda in0, in1, s0, s1, imm2: np.maximum(
            in0.astype(np.float32) * s0 + in1, in1
        ),
    )
    row = max(dve_ops._SUB_OPCODE_FOR_NAME.values()) + 1
    assert row < 0x20
    dve_ops._SUB_OPCODE_FOR_NAME[name] = row
    shas = {}
    for ver in ("v3", "v4"):
        tmp = DveOpSpec(name=name, opcode=row, uops=lower(spec, ver=ver),
                        rd1_en=_has_src1(spec))
        shas[ver] = tmp.sha(ver)
    op = dve_ops.DveOp(name, spec, subdim=False, uops_sha=shas)
    dve_ops.OPS.append(op)
    dve_ops.CUSTOM_DVE_SPECS[name] = spec
    _RELU_MAC = op
    return op


def _build(allowed):
    """Trace + compile the per-core kernel, specialized on the allowed
    shared-expert sets (from sim_domain)."""
    import concourse.bacc as bacc
    import concourse.bass as bass
    import concourse.mybir as mybir
    import concourse.tile as tile

    RELU_MAC = _get_relu_mac()

    f32 = mybir.dt.float32
    bf16 = mybir.dt.bfloat16
    Alu = mybir.AluOpType
    Act = mybir.ActivationFunctionType
    Ax = mybir.AxisListType

    nc = bacc.Bacc(None, target_bir_lowering=False, debug=False)

    xt = nc.declare_dram_parameter("xt", [5, DIN, BC], bf16, isOutput=False)
    wsp = nc.declare_dram_parameter("wsp", [D_NUM, DIN, N_ES * H], bf16, isOutput=False)
    wsh = nc.declare_dram_parameter("wsh", [DIN, N_SH * H], bf16, isOutput=False)
    wg = nc.declare_dram_parameter("wg", [DIN, D_NUM * 6], bf16, isOutput=False)
    wgs = nc.declare_dram_parameter("wgs", [DIN, 12], bf16, isOutput=False)
    bias = nc.declare_dram_parameter("bias", [32, NB], bf16, isOutput=False)
    ones = nc.declare_dram_parameter("ones", [32, 128], bf16, isOutput=False)
    bmask = nc.declare_dram_parameter("bmask", [128, D_NUM, GRP, 6], f32, isOutput=False)
    out = nc.declare_dram_parameter("out", [5, BC, H], f32, isOutput=True)

    with tile.TileContext(nc) as tc:
        with (
            tc.tile_pool(name="wpool", bufs=1) as wp,
            tc.tile_pool(name="xpool", bufs=2) as xp,
            tc.tile_pool(name="ogpool", bufs=2) as ogp,
            tc.tile_pool(name="smpool", bufs=3) as sp,
            tc.tile_pool(name="scrpool", bufs=10) as scp,
            tc.tile_pool(name="pbig", bufs=6, space=bass.MemorySpace.PSUM) as pb,
            tc.tile_pool(name="pgate", bufs=2, space=bass.MemorySpace.PSUM) as pg,
        ):
            # ---- persistent weights (small/gate tensors first for fast start) ----
            wg_sb = wp.tile([128, KC, D_NUM * 6], bf16, tag="wg")
            nc.sync.dma_start(wg_sb[:], wg.rearrange("(c p) n -> p c n", p=128))
            wgs_sb = wp.tile([128, KC, 12], bf16, tag="wgs")
            nc.sync.dma_start(wgs_sb[:], wgs.rearrange("(c p) n -> p c n", p=128))
            bias_sb = wp.tile([32, NB], bf16, tag="bias")
            nc.sync.dma_start(bias_sb[:], bias[:])
            ones_sb = wp.tile([32, 128], bf16, tag="ones")
            nc.sync.dma_start(ones_sb[:], ones[:])
            bmask_sb = wp.tile([128, D_NUM, GRP, 6], f32, tag="bmask")
            nc.sync.dma_start(bmask_sb[:], bmask[:])
            wsp_sb = wp.tile([128, D_NUM, KC, N_ES * H], bf16, tag="wsp")
            for d in range(D_NUM):
                nc.sync.dma_start(wsp_sb[:, d], wsp[d].rearrange("(c p) n -> p c n", p=128))
            wsh_sb = wp.tile([128, KC, N_SH * H], bf16, tag="wsh")
            nc.sync.dma_start(wsh_sb[:], wsh.rearrange("(c p) n -> p c n", p=128))

            def alloc_xtg(g):
                j0 = g * (GRP * 128)
                xtg = xp.tile([128, 5, KC, GRP * 128], bf16, tag="xtg", name=f"xtg{g}")
                nc.sync.dma_start(
                    xtg[:],
                    xt[:, :, j0 : j0 + GRP * 128].rearrange("i (c p) j -> p i c j", p=128),
                )
                return xtg

            def gate_mm_emitters(g, xtg, gbank):
                """List of closures, one per gate matmul (bias first)."""
                ems = [lambda: nc.tensor.matmul(
                    gbank[:], ones_sb[:, :], bias_sb[:, OFF_GB:NB],
                    start=True, stop=False, skip_group_check=True)]
                for t in range(GRP):
                    for d in range(D_NUM):
                        o0 = (d * GRP + t) * 6
                        for c in range(KC):
                            ems.append(lambda t=t, d=d, c=c, o0=o0: nc.tensor.matmul(
                                gbank[:, o0 : o0 + 6],
                                xtg[:, d, c, t * 128 : (t + 1) * 128],
                                wg_sb[:, c, 6 * d : 6 * d + 6],
                                start=False, stop=False, skip_group_check=True))
                    o1 = GB_GS + t * 12
                    for c in range(KC):
                        last = (t == GRP - 1 and c == KC - 1)
                        ems.append(lambda t=t, c=c, o1=o1, last=last: nc.tensor.matmul(
                            gbank[:, o1 : o1 + 12],
                            xtg[:, 4, c, t * 128 : (t + 1) * 128],
                            wgs_sb[:, c, :],
                            start=False, stop=last, skip_group_check=True))
                return ems

            def emit_softmax(g, gbank):
                gview = gbank[:, 0:GB_GS].rearrange("p (d t s) -> p d t s", d=D_NUM, t=GRP)
                e1 = sp.tile([128, D_NUM, GRP, 6], f32, tag="e1", name=f"e1_{g}")
                nc.scalar.activation(e1[:], gview, Act.Exp)
                s1 = sp.tile([128, D_NUM, GRP], f32, tag="s1", name=f"s1_{g}")
                nc.vector.tensor_reduce(s1[:], e1[:], axis=Ax.X, op=Alu.add)
                r1 = sp.tile([128, D_NUM, GRP], f32, tag="r1", name=f"r1_{g}")
                nc.vector.reciprocal(r1[:], s1[:])
                gn = sp.tile([128, D_NUM, GRP, 6], f32, tag="gn", name=f"gn_{g}")
                nc.vector.tensor_tensor(
                    gn[:], e1[:], r1[:, :, :, None].to_broadcast([128, D_NUM, GRP, 6]),
                    Alu.mult)
                e2 = sp.tile([128, D_NUM, GRP, 6], f32, tag="e2", name=f"e2_{g}")
                nc.scalar.activation(e2[:], gn[:], Act.Exp)
                e2m = sp.tile([128, D_NUM, GRP, 6], f32, tag="e2m", name=f"e2m_{g}")
                nc.vector.tensor_tensor(e2m[:], e2[:], bmask_sb[:], Alu.mult)
                s2 = sp.tile([128, D_NUM, GRP], f32, tag="s2", name=f"s2_{g}")
                nc.vector.tensor_reduce(s2[:], e2m[:], axis=Ax.X, op=Alu.add)
                r2 = sp.tile([128, D_NUM, GRP], f32, tag="r2", name=f"r2_{g}")
                nc.vector.reciprocal(r2[:], s2[:])
                g2 = sp.tile([128, D_NUM, GRP, 6], f32, tag="g2", name=f"g2_{g}")
                nc.vector.tensor_tensor(
                    g2[:], e2m[:], r2[:, :, :, None].to_broadcast([128, D_NUM, GRP, 6]),
                    Alu.mult)

                gsview = gbank[:, GB_GS:NGB].rearrange("p (t s) -> p t s", t=GRP)
                egs = sp.tile([128, GRP, 12], f32, tag="egs", name=f"egs{g}")
                nc.scalar.activation(egs[:], gsview, Act.Exp)
                sgs = sp.tile([128, GRP], f32, tag="sgs", name=f"sgs{g}")
                nc.vector.tensor_reduce(sgs[:], egs[:], axis=Ax.X, op=Alu.add)
                rgs = sp.tile([128, GRP], f32, tag="rgs", name=f"rgs{g}")
                nc.vector.reciprocal(rgs[:], sgs[:])
                gs = sp.tile([128, GRP, 12], f32, tag="gs", name=f"gs{g}")
                nc.vector.tensor_tensor(
                    gs[:], egs[:], rgs[:, :, None].to_broadcast([128, GRP, 12]), Alu.mult)
                return g2, gs

            def emit_tile(g, t, xtg, og, g2, gs, gate_block):
                """Expert matmuls + combines for one 128-row tile; gate_block is
                a list of next-group gate-MM emitters to weave into the stream."""
                j0 = g * (GRP * 128)
                og_s = og[:, 4, t, :]
                gi = iter(gate_block)

                def weave(n):
                    for _ in range(n):
                        em = next(gi, None)
                        if em is not None:
                            em()

                psd = []
                for d in range(D_NUM):
                    ps = pb.tile([128, 512], f32, tag="pb", name=f"ps{g}_{t}_{d}")
                    psd.append(ps)
                    nc.tensor.matmul(ps[:], ones_sb[:, :],
                                     bias_sb[:, 512 * d : 512 * d + 512],
                                     start=True, stop=False)
                    for c in range(KC):
                        nc.tensor.matmul(ps[:], xtg[:, d, c, t * 128 : (t + 1) * 128],
                                         wsp_sb[:, d, c, :], start=False,
                                         stop=(c == KC - 1))
                        weave(1)
                for d in range(D_NUM):
                    ps = psd[d]
                    og_d = og[:, d, t, :]
                    nc.scalar.activation(og_d, ps[:, 0:H], Act.Relu,
                                         scale=g2[:, d, t, 0:1])
                    nc.vector._custom_dve(RELU_MAC, out=og_d, in0=ps[:, H : 2 * H],
                                          in1=og_d, s0=g2[:, d, t, 1:2])
                    if d == 0:
                        nc.scalar.activation(og_s, ps[:, 0:H], Act.Relu,
                                             scale=gs[:, t, 0:1])
                    else:
                        nc.vector._custom_dve(RELU_MAC, out=og_s, in0=ps[:, 0:H],
                                              in1=og_s, s0=gs[:, t, 2 * d : 2 * d + 1])
                    nc.vector._custom_dve(RELU_MAC, out=og_s, in0=ps[:, H : 2 * H],
                                          in1=og_s, s0=gs[:, t, 2 * d + 1 : 2 * d + 2])

                ps_a = pb.tile([128, 512], f32, tag="pb", name=f"psa{g}_{t}")
                nc.tensor.matmul(ps_a[:], ones_sb[:, :],
                                 bias_sb[:, OFF_SH : OFF_SH + 512],
                                 start=True, stop=False)
                for c in range(KC):
                    nc.tensor.matmul(ps_a[:], xtg[:, 4, c, t * 128 : (t + 1) * 128],
                                     wsh_sb[:, c, 0:512], start=False,
                                     stop=(c == KC - 1))
                    weave(1)
                ps_b = pb.tile([128, 512], f32, tag="pb", name=f"psb{g}_{t}")
                nc.tensor.matmul(ps_b[:], ones_sb[:, :],
                                 bias_sb[:, OFF_SH + 512 : OFF_SH + 1024],
                                 start=True, stop=False)
                for c in range(KC):
                    nc.tensor.matmul(ps_b[:], xtg[:, 4, c, t * 128 : (t + 1) * 128],
                                     wsh_sb[:, c, 512:1024], start=False,
                                     stop=(c == KC - 1))
                    weave(1)
                weave(len(gate_block))  # flush any leftovers in this block
                shp_ = {0: (ps_a, 0), 1: (ps_a, H), 2: (ps_b, 0), 3: (ps_b, H)}

                for d in range(D_NUM):
                    og_d = og[:, d, t, :]
                    for s in allowed[d]:
                        bank, off = shp_[s]
                        nc.vector._custom_dve(RELU_MAC, out=og_d,
                                              in0=bank[:, off : off + H],
                                              in1=og_d, s0=g2[:, d, t, 2 + s : 3 + s])
                for s in range(N_SH):
                    bank, off = shp_[s]
                    scr = scp.tile([128, H], bf16, tag="scr", name=f"scr{g}_{t}_{s}")
                    nc.scalar.activation(scr[:], bank[:, off : off + H], Act.Relu,
                                         scale=gs[:, t, 8 + s : 9 + s])
                    nc.gpsimd.tensor_tensor(og_s, og_s, scr[:], Alu.add)

                r0 = j0 + t * 128
                nc.gpsimd.dma_start(
                    out[:, r0 : r0 + 128, :].rearrange("i p h -> p i h"),
                    og[:, :, t, :])

            # ---- software pipeline over groups ----
            xtg_cur = alloc_xtg(0)
            gbank_cur = pg.tile([128, NGB], f32, tag="pg", name="gb0")
            for em in gate_mm_emitters(0, xtg_cur, gbank_cur):
                em()
            sm_cur = emit_softmax(0, gbank_cur)

            for g in range(NG):
                og = ogp.tile([128, 5, GRP, H], bf16, tag="og", name=f"og{g}")
                if g + 1 < NG:
                    xtg_next = alloc_xtg(g + 1)
                    gbank_next = pg.tile([128, NGB], f32, tag="pg", name=f"gb{g+1}")
                    ems = gate_mm_emitters(g + 1, xtg_next, gbank_next)
                    # split gate MMs across this group's tiles (skip tile 0 so the
                    # next group's xtg DMA has time to land)
                    nblk = GRP - 1
                    per = (len(ems) + nblk - 1) // nblk
                    blocks = [[] ] + [ems[i * per : (i + 1) * per] for i in range(nblk)]
                else:
                    blocks = [[] for _ in range(GRP)]
                for t in range(GRP):
                    emit_tile(g, t, xtg_cur, og, sm_cur[0], sm_cur[1], blocks[t])
                if g + 1 < NG:
                    sm_cur = emit_softmax(g + 1, gbank_next)
                    xtg_cur = xtg_next

    nc.compile()
    return nc


def _prep_inputs(inputs):
    """Host-side shard + relayout. Returns (in_maps, allowed)."""
    import ml_dtypes
    bf16_np = ml_dtypes.bfloat16

    x_list = np.asarray(inputs["x_list"], dtype=np.float32)
    sim_domain = np.asarray(inputs["sim_domain"])
    W_spec = np.asarray(inputs["W_spec"], dtype=np.float32)
    b_spec = np.asarray(inputs["b_spec"], dtype=np.float32)
    W_sh = np.asarray(inputs["W_sh"], dtype=np.float32)
    b_sh = np.asarray(inputs["b_sh"], dtype=np.float32)
    W_gate = np.asarray(inputs["W_gate"], dtype=np.float32)
    b_gate = np.asarray(inputs["b_gate"], dtype=np.float32)
    W_gate_sh = np.asarray(inputs["W_gate_sh"], dtype=np.float32)
    b_gate_sh = np.asarray(inputs["b_gate_sh"], dtype=np.float32)

    mem = (sim_domain[:, :, None] == np.arange(D_NUM)[None, None, :]).any(axis=1)
    allowed = tuple(tuple(int(s) for s in range(N_SH) if mem[d, s]) for d in range(D_NUM))

    wsp = np.ascontiguousarray(
        W_spec.transpose(0, 2, 1, 3).reshape(D_NUM, DIN, N_ES * H)
    ).astype(bf16_np)
    wsh = np.ascontiguousarray(W_sh.transpose(1, 0, 2).reshape(DIN, N_SH * H)).astype(bf16_np)
    wg = np.ascontiguousarray(W_gate.transpose(1, 0, 2).reshape(DIN, D_NUM * 6)).astype(bf16_np)
    wgs = np.ascontiguousarray(W_gate_sh).astype(bf16_np)

    gb_bias = np.concatenate(
        [np.repeat(b_gate[:, None, :], GRP, axis=1).reshape(-1),
         np.tile(b_gate_sh, GRP)]
    )
    bias_row = np.concatenate(
        [b_spec.reshape(D_NUM * N_ES * H), b_sh.reshape(N_SH * H),
         np.zeros(36, np.float32), gb_bias]
    ).astype(np.float32)
    assert bias_row.shape[0] == NB
    bias = np.zeros((32, NB), np.float32)
    bias[0] = bias_row
    bias = bias.astype(bf16_np)
    ones = np.zeros((32, 128), np.float32)
    ones[0] = 1.0
    ones = ones.astype(bf16_np)

    bmask_row = np.ones((D_NUM, 6), np.float32)
    bmask_row[:, N_ES:] = mem.astype(np.float32)
    bmask = np.broadcast_to(
        np.repeat(bmask_row[None, :, None, :], GRP, axis=2), (128, D_NUM, GRP, 6)
    ).copy()

    shared = {"wsp": wsp, "wsh": wsh, "wg": wg, "wgs": wgs,
              "bias": bias, "ones": ones, "bmask": bmask}
    in_maps = []
    for c in range(N_CORES):
        sl = x_list[:, c * BC : (c + 1) * BC, :]
        xt_c = np.ascontiguousarray(sl.transpose(0, 2, 1)).astype(bf16_np)
        in_maps.append({"xt": xt_c, **shared})
    return in_maps, allowed


def _run(inputs, trace=False, trace_kwargs=None):
    from concourse.bass_utils import run_bass_kernel_spmd

    in_maps, allowed = _prep_inputs(inputs)
    key = allowed
    if key not in _BUILD_CACHE:
        _BUILD_CACHE[key] = _build(allowed)
    nc = _BUILD_CACHE[key]

    kw = {}
    if trace:
        kw["trace"] = True
        if trace_kwargs:
            kw.update(trace_kwargs)
    res = run_bass_kernel_spmd(nc, in_maps, list(range(N_CORES)), **kw)
    full = np.empty((5, B, H), np.float32)
    for c in range(N_CORES):
        full[:, c * BC : (c + 1) * BC, :] = res.results[c]["out"]
    return full, res


def kernel(**inputs):
    full, _ = _run(inputs)
    return full


# revision 7
# speedup vs baseline: 1.1006x; 1.1006x over previous
"""Trainium2 Bass kernel for the CGC (Customized Gate Control) MoE routing module.

Contract: kernel(**inputs) takes the FULL unsharded inputs (numpy/jax arrays)
and returns the FULL output [5, 16384, 256] float32.

Strategy:
  - Data-parallel over batch across 8 NeuronCores (2048 rows/core).
  - Host prep: per-core x slices fed pre-transposed [DIN, B_c] (contraction dim
    on SBUF partitions, fully contiguous DMAs, no on-device transposes);
    weights replicated, packed [DIN, E*H], cast to bf16 (fp32 matmul is a
    2-pass HI/LO op on the TRN2 PE - half throughput). PSUM stays fp32.
  - Expert biases enter PSUM via a K=32 one-hot matmul.
  - All gate logits for a group live in ONE PSUM bank (one bias matmul +
    region-wise accumulation); the gate matmuls of group g+1 are interleaved
    into group g's expert matmul stream so their LDWEIGHTS hide under the
    512-column expert matmuls and the PE never idles (HAM stays at 8/8).
  - Softmax runs batched over all domains x tiles of a group, with broadcast
    (step-0) tensor_tensor ops for the normalizations.
  - The gated combine uses a runtime-registered custom DVE op RELU_MAC:
        out = max(in0*s0 + in1, in1)  ==  s0*relu(in0) + in1   (s0 >= 0)
    reading expert PSUM banks directly - no relu evictions to SBUF. First
    terms are fused into ScalarE scaled-relu evictions; the out_sh
    shared-expert adds run on GpSimd to offload the Vector engine.
  - The double-softmax mask is known from sim_domain at trace time; masked
    shared-expert terms are not emitted (kernel is compile-specialized).
"""

import sys

sys.path.insert(0, "/opt/trn_rl_repo")

import numpy as np

D_NUM = 4
N_ES = 2
N_SH = 4
DIN = 512
H = 256
B = 16384
N_CORES = 8
BC = B // N_CORES          # 2048 rows per core
KC = DIN // 128            # 4 contraction chunks
GRP = 8                    # batch tiles (of 128 rows) per group
NG = BC // (128 * GRP)     # groups per core

# bias row layout: [spec d*512 | shared 1024 | (pad 36) | gate-bank]
OFF_SH = D_NUM * 512                     # 2048
OFF_GB = OFF_SH + N_SH * H + 36          # 3108 gate-bank bias region
GB_GS = D_NUM * GRP * 6                  # gsh region offset inside gate bank
NGB = GB_GS + GRP * 12
NB = OFF_GB + NGB

_BUILD_CACHE = {}
_RELU_MAC = None


def _get_relu_mac():
    """Register the RELU_MAC custom DVE op (idempotent)."""
    global _RELU_MAC
    if _RELU_MAC is not None:
        return _RELU_MAC
    from concourse import dve_ops
    from concourse.dve_spec import Spec, Src0, Src1, C0, maxx, lower, _has_src1
    from concourse.dve_uop import DveOpSpec

    name = "RELU_MAC_ANT"
    for o in dve_ops.OPS:
        if o.name == name:
            _RELU_MAC = o
            return o
    spec = Spec(
        body=maxx(Src0 * C0 + Src1, Src1),
        reference=lambda in0, in1, s0, s1, imm2: np.maximum(
            in0.astype(np.float32) * s0 + in1, in1
        ),
    )
    row = max(dve_ops._SUB_OPCODE_FOR_NAME.values()) + 1
    assert row < 0x20
    dve_ops._SUB_OPCODE_FOR_NAME[name] = row
    shas = {}
    for ver in ("v3", "v4"):
        tmp = DveOpSpec(name=name, opcode=row, uops=lower(spec, ver=ver),
                        rd1_en=_has_src1(spec))
        shas[ver] = tmp.sha(ver)
    op = dve_ops.DveOp(name, spec, subdim=False, uops_sha=shas)
    dve_ops.OPS.append(op)
    dve_ops.CUSTOM_DVE_SPECS[name] = spec
    _RELU_MAC = op
    return op


def _build(allowed):
    """Trace + compile the per-core kernel, specialized on the allowed
    shared-expert sets (from sim_domain)."""
    import concourse.bacc as bacc
    import concourse.bass as bass
    import concourse.mybir as mybir
    import concourse.tile as tile

    RELU_MAC = _get_relu_mac()

    f32 = mybir.dt.float32
    bf16 = mybir.dt.bfloat16
    Alu = mybir.AluOpType
    Act = mybir.ActivationFunctionType
    Ax = mybir.AxisListType

    nc = bacc.Bacc(None, target_bir_lowering=False, debug=False)

    xt = nc.declare_dram_parameter("xt", [5, DIN, BC], bf16, isOutput=False)
    wsp = nc.declare_dram_parameter("wsp", [D_NUM, DIN, N_ES * H], bf16, isOutput=False)
    wsh = nc.declare_dram_parameter("wsh", [DIN, N_SH * H], bf16, isOutput=False)
    wg = nc.declare_dram_parameter("wg", [DIN, D_NUM * 6], bf16, isOutput=False)
    wgs = nc.declare_dram_parameter("wgs", [DIN, 12], bf16, isOutput=False)
    bias = nc.declare_dram_parameter("bias", [32, NB], bf16, isOutput=False)
    ones = nc.declare_dram_parameter("ones", [32, 128], bf16, isOutput=False)
    bmask = nc.declare_dram_parameter("bmask", [128, D_NUM, GRP, 6], f32, isOutput=False)
    out = nc.declare_dram_parameter("out", [5, BC, H], f32, isOutput=True)

    with tile.TileContext(nc) as tc:
        with (
            tc.tile_pool(name="wpool", bufs=1) as wp,
            tc.tile_pool(name="xpool", bufs=2) as xp,
            tc.tile_pool(name="ogpool", bufs=2) as ogp,
            tc.tile_pool(name="smpool", bufs=3) as sp,
            tc.tile_pool(name="scrpool", bufs=10) as scp,
            tc.tile_pool(name="pbig", bufs=6, space=bass.MemorySpace.PSUM) as pb,
            tc.tile_pool(name="pgate", bufs=2, space=bass.MemorySpace.PSUM) as pg,
        ):
            # ---- persistent weights; xt of group 0 + gate weights first so the
            # gate phase starts while the big expert weights stream in ----
            xtg0 = xp.tile([128, 5, KC, GRP * 128], bf16, tag="xtg", name="xtg0")
            nc.sync.dma_start(
                xtg0[:], xt[:, :, 0 : GRP * 128].rearrange("i (c p) j -> p i c j", p=128))
            wg_sb = wp.tile([128, KC, D_NUM * 6], bf16, tag="wg")
            nc.sync.dma_start(wg_sb[:], wg.rearrange("(c p) n -> p c n", p=128))
            wgs_sb = wp.tile([128, KC, 12], bf16, tag="wgs")
            nc.sync.dma_start(wgs_sb[:], wgs.rearrange("(c p) n -> p c n", p=128))
            bias_sb = wp.tile([32, NB], bf16, tag="bias")
            nc.sync.dma_start(bias_sb[:], bias[:])
            ones_sb = wp.tile([32, 128], bf16, tag="ones")
            nc.sync.dma_start(ones_sb[:], ones[:])
            bmask_sb = wp.tile([128, D_NUM, GRP, 6], f32, tag="bmask")
            nc.sync.dma_start(bmask_sb[:], bmask[:])
            wsp_sb = wp.tile([128, D_NUM, KC, N_ES * H], bf16, tag="wsp")
            for d in range(D_NUM):
                nc.sync.dma_start(wsp_sb[:, d], wsp[d].rearrange("(c p) n -> p c n", p=128))
            wsh_sb = wp.tile([128, KC, N_SH * H], bf16, tag="wsh")
            nc.sync.dma_start(wsh_sb[:], wsh.rearrange("(c p) n -> p c n", p=128))

            def alloc_xtg(g):
                if g == 0:
                    return xtg0
                j0 = g * (GRP * 128)
                xtg = xp.tile([128, 5, KC, GRP * 128], bf16, tag="xtg", name=f"xtg{g}")
                nc.sync.dma_start(
                    xtg[:],
                    xt[:, :, j0 : j0 + GRP * 128].rearrange("i (c p) j -> p i c j", p=128),
                )
                return xtg

            def gate_mm_emitters(g, xtg, gbank):
                """List of closures, one per gate matmul (bias first)."""
                ems = [lambda: nc.tensor.matmul(
                    gbank[:], ones_sb[:, :], bias_sb[:, OFF_GB:NB],
                    start=True, stop=False, skip_group_check=True)]
                for t in range(GRP):
                    for d in range(D_NUM):
                        o0 = (d * GRP + t) * 6
                        for c in range(KC):
                            ems.append(lambda t=t, d=d, c=c, o0=o0: nc.tensor.matmul(
                                gbank[:, o0 : o0 + 6],
                                xtg[:, d, c, t * 128 : (t + 1) * 128],
                                wg_sb[:, c, 6 * d : 6 * d + 6],
                                start=False, stop=False, skip_group_check=True))
                    o1 = GB_GS + t * 12
                    for c in range(KC):
                        last = (t == GRP - 1 and c == KC - 1)
                        ems.append(lambda t=t, c=c, o1=o1, last=last: nc.tensor.matmul(
                            gbank[:, o1 : o1 + 12],
                            xtg[:, 4, c, t * 128 : (t + 1) * 128],
                            wgs_sb[:, c, :],
                            start=False, stop=last, skip_group_check=True))
                return ems

            def emit_softmax(g, gbank):
                gview = gbank[:, 0:GB_GS].rearrange("p (d t s) -> p d t s", d=D_NUM, t=GRP)
                e1 = sp.tile([128, D_NUM, GRP, 6], f32, tag="e1", name=f"e1_{g}")
                nc.scalar.activation(e1[:], gview, Act.Exp)
                s1 = sp.tile([128, D_NUM, GRP], f32, tag="s1", name=f"s1_{g}")
                nc.vector.tensor_reduce(s1[:], e1[:], axis=Ax.X, op=Alu.add)
                r1 = sp.tile([128, D_NUM, GRP], f32, tag="r1", name=f"r1_{g}")
                nc.vector.reciprocal(r1[:], s1[:])
                gn = sp.tile([128, D_NUM, GRP, 6], f32, tag="gn", name=f"gn_{g}")
                nc.vector.tensor_tensor(
                    gn[:], e1[:], r1[:, :, :, None].to_broadcast([128, D_NUM, GRP, 6]),
                    Alu.mult)
                e2 = sp.tile([128, D_NUM, GRP, 6], f32, tag="e2", name=f"e2_{g}")
                nc.scalar.activation(e2[:], gn[:], Act.Exp)
                e2m = sp.tile([128, D_NUM, GRP, 6], f32, tag="e2m", name=f"e2m_{g}")
                nc.vector.tensor_tensor(e2m[:], e2[:], bmask_sb[:], Alu.mult)
                s2 = sp.tile([128, D_NUM, GRP], f32, tag="s2", name=f"s2_{g}")
                nc.vector.tensor_reduce(s2[:], e2m[:], axis=Ax.X, op=Alu.add)
                r2 = sp.tile([128, D_NUM, GRP], f32, tag="r2", name=f"r2_{g}")
                nc.vector.reciprocal(r2[:], s2[:])
                g2 = sp.tile([128, D_NUM, GRP, 6], f32, tag="g2", name=f"g2_{g}")
                nc.vector.tensor_tensor(
                    g2[:], e2m[:], r2[:, :, :, None].to_broadcast([128, D_NUM, GRP, 6]),
                    Alu.mult)

                gsview = gbank[:, GB_GS:NGB].rearrange("p (t s) -> p t s", t=GRP)
                egs = sp.tile([128, GRP, 12], f32, tag="egs", name=f"egs{g}")
                nc.scalar.activation(egs[:], gsview, Act.Exp)
                sgs = sp.tile([128, GRP], f32, tag="sgs", name=f"sgs{g}")
                nc.vector.tensor_reduce(sgs[:], egs[:], axis=Ax.X, op=Alu.add)
                rgs = sp.tile([128, GRP], f32, tag="rgs", name=f"rgs{g}")
                nc.vector.reciprocal(rgs[:], sgs[:])
                gs = sp.tile([128, GRP, 12], f32, tag="gs", name=f"gs{g}")
                nc.vector.tensor_tensor(
                    gs[:], egs[:], rgs[:, :, None].to_broadcast([128, GRP, 12]), Alu.mult)
                return g2, gs

            def emit_tile(g, t, xtg, og, g2, gs, gate_block):
                """Expert matmuls + combines for one 128-row tile; gate_block is
                a list of next-group gate-MM emitters to weave into the stream."""
                j0 = g * (GRP * 128)
                og_s = og[:, 4, t, :]
                gi = iter(gate_block)

                def weave(n):
                    for _ in range(n):
                        em = next(gi, None)
                        if em is not None:
                            em()

                psd = []
                for d in range(D_NUM):
                    ps = pb.tile([128, 512], f32, tag="pb", name=f"ps{g}_{t}_{d}")
                    psd.append(ps)
                    nc.tensor.matmul(ps[:], ones_sb[:, :],
                                     bias_sb[:, 512 * d : 512 * d + 512],
                                     start=True, stop=False)
                    for c in range(KC):
                        nc.tensor.matmul(ps[:], xtg[:, d, c, t * 128 : (t + 1) * 128],
                                         wsp_sb[:, d, c, :], start=False,
                                         stop=(c == KC - 1))
                        weave(1)
                for d in range(D_NUM):
                    ps = psd[d]
                    og_d = og[:, d, t, :]
                    nc.scalar.activation(og_d, ps[:, 0:H], Act.Relu,
                                         scale=g2[:, d, t, 0:1])
                    nc.vector._custom_dve(RELU_MAC, out=og_d, in0=ps[:, H : 2 * H],
                                          in1=og_d, s0=g2[:, d, t, 1:2])
                    if d == 0:
                        nc.scalar.activation(og_s, ps[:, 0:H], Act.Relu,
                                             scale=gs[:, t, 0:1])
                    else:
                        nc.vector._custom_dve(RELU_MAC, out=og_s, in0=ps[:, 0:H],
                                              in1=og_s, s0=gs[:, t, 2 * d : 2 * d + 1])
                    nc.vector._custom_dve(RELU_MAC, out=og_s, in0=ps[:, H : 2 * H],
                                          in1=og_s, s0=gs[:, t, 2 * d + 1 : 2 * d + 2])

                ps_a = pb.tile([128, 512], f32, tag="pb", name=f"psa{g}_{t}")
                nc.tensor.matmul(ps_a[:], ones_sb[:, :],
                                 bias_sb[:, OFF_SH : OFF_SH + 512],
                                 start=True, stop=False)
                for c in range(KC):
                    nc.tensor.matmul(ps_a[:], xtg[:, 4, c, t * 128 : (t + 1) * 128],
                                     wsh_sb[:, c, 0:512], start=False,
                                     stop=(c == KC - 1))
                    weave(1)
                ps_b = pb.tile([128, 512], f32, tag="pb", name=f"psb{g}_{t}")
                nc.tensor.matmul(ps_b[:], ones_sb[:, :],
                                 bias_sb[:, OFF_SH + 512 : OFF_SH + 1024],
                                 start=True, stop=False)
                for c in range(KC):
                    nc.tensor.matmul(ps_b[:], xtg[:, 4, c, t * 128 : (t + 1) * 128],
                                     wsh_sb[:, c, 512:1024], start=False,
                                     stop=(c == KC - 1))
                    weave(1)
                weave(len(gate_block))  # flush any leftovers in this block
                shp_ = {0: (ps_a, 0), 1: (ps_a, H), 2: (ps_b, 0), 3: (ps_b, H)}

                for d in range(D_NUM):
                    og_d = og[:, d, t, :]
                    for s in allowed[d]:
                        bank, off = shp_[s]
                        nc.vector._custom_dve(RELU_MAC, out=og_d,
                                              in0=bank[:, off : off + H],
                                              in1=og_d, s0=g2[:, d, t, 2 + s : 3 + s])
                for s in range(N_SH):
                    bank, off = shp_[s]
                    scr = scp.tile([128, H], bf16, tag="scr", name=f"scr{g}_{t}_{s}")
                    nc.scalar.activation(scr[:], bank[:, off : off + H], Act.Relu,
                                         scale=gs[:, t, 8 + s : 9 + s])
                    nc.gpsimd.tensor_tensor(og_s, og_s, scr[:], Alu.add)

                r0 = j0 + t * 128
                nc.gpsimd.dma_start(
                    out[:, r0 : r0 + 128, :].rearrange("i p h -> p i h"),
                    og[:, :, t, :])

            # ---- software pipeline over groups ----
            xtg_cur = alloc_xtg(0)
            gbank_cur = pg.tile([128, NGB], f32, tag="pg", name="gb0")
            for em in gate_mm_emitters(0, xtg_cur, gbank_cur):
                em()
            sm_cur = emit_softmax(0, gbank_cur)

            for g in range(NG):
                og = ogp.tile([128, 5, GRP, H], bf16, tag="og", name=f"og{g}")
                if g + 1 < NG:
                    xtg_next = alloc_xtg(g + 1)
                    gbank_next = pg.tile([128, NGB], f32, tag="pg", name=f"gb{g+1}")
                    ems = gate_mm_emitters(g + 1, xtg_next, gbank_next)
                    # split gate MMs across this group's tiles (skip tile 0 so the
                    # next group's xtg DMA has time to land)
                    nblk = GRP - 1
                    per = (len(ems) + nblk - 1) // nblk
                    blocks = [[] ] + [ems[i * per : (i + 1) * per] for i in range(nblk)]
                else:
                    blocks = [[] for _ in range(GRP)]
                for t in range(GRP):
                    emit_tile(g, t, xtg_cur, og, sm_cur[0], sm_cur[1], blocks[t])
                if g + 1 < NG:
                    sm_cur = emit_softmax(g + 1, gbank_next)
                    xtg_cur = xtg_next

    nc.compile()
    return nc


def _prep_inputs(inputs):
    """Host-side shard + relayout. Returns (in_maps, allowed)."""
    import ml_dtypes
    bf16_np = ml_dtypes.bfloat16

    x_list = np.asarray(inputs["x_list"], dtype=np.float32)
    sim_domain = np.asarray(inputs["sim_domain"])
    W_spec = np.asarray(inputs["W_spec"], dtype=np.float32)
    b_spec = np.asarray(inputs["b_spec"], dtype=np.float32)
    W_sh = np.asarray(inputs["W_sh"], dtype=np.float32)
    b_sh = np.asarray(inputs["b_sh"], dtype=np.float32)
    W_gate = np.asarray(inputs["W_gate"], dtype=np.float32)
    b_gate = np.asarray(inputs["b_gate"], dtype=np.float32)
    W_gate_sh = np.asarray(inputs["W_gate_sh"], dtype=np.float32)
    b_gate_sh = np.asarray(inputs["b_gate_sh"], dtype=np.float32)

    mem = (sim_domain[:, :, None] == np.arange(D_NUM)[None, None, :]).any(axis=1)
    allowed = tuple(tuple(int(s) for s in range(N_SH) if mem[d, s]) for d in range(D_NUM))

    wsp = np.ascontiguousarray(
        W_spec.transpose(0, 2, 1, 3).reshape(D_NUM, DIN, N_ES * H)
    ).astype(bf16_np)
    wsh = np.ascontiguousarray(W_sh.transpose(1, 0, 2).reshape(DIN, N_SH * H)).astype(bf16_np)
    wg = np.ascontiguousarray(W_gate.transpose(1, 0, 2).reshape(DIN, D_NUM * 6)).astype(bf16_np)
    wgs = np.ascontiguousarray(W_gate_sh).astype(bf16_np)

    gb_bias = np.concatenate(
        [np.repeat(b_gate[:, None, :], GRP, axis=1).reshape(-1),
         np.tile(b_gate_sh, GRP)]
    )
    bias_row = np.concatenate(
        [b_spec.reshape(D_NUM * N_ES * H), b_sh.reshape(N_SH * H),
         np.zeros(36, np.float32), gb_bias]
    ).astype(np.float32)
    assert bias_row.shape[0] == NB
    bias = np.zeros((32, NB), np.float32)
    bias[0] = bias_row
    bias = bias.astype(bf16_np)
    ones = np.zeros((32, 128), np.float32)
    ones[0] = 1.0
    ones = ones.astype(bf16_np)

    bmask_row = np.ones((D_NUM, 6), np.float32)
    bmask_row[:, N_ES:] = mem.astype(np.float32)
    bmask = np.broadcast_to(
        np.repeat(bmask_row[None, :, None, :], GRP, axis=2), (128, D_NUM, GRP, 6)
    ).copy()

    shared = {"wsp": wsp, "wsh": wsh, "wg": wg, "wgs": wgs,
              "bias": bias, "ones": ones, "bmask": bmask}
    in_maps = []
    for c in range(N_CORES):
        sl = x_list[:, c * BC : (c + 1) * BC, :]
        xt_c = np.ascontiguousarray(sl.transpose(0, 2, 1)).astype(bf16_np)
        in_maps.append({"xt": xt_c, **shared})
    return in_maps, allowed


def _run(inputs, trace=False, trace_kwargs=None):
    from concourse.bass_utils import run_bass_kernel_spmd

    in_maps, allowed = _prep_inputs(inputs)
    key = allowed
    if key not in _BUILD_CACHE:
        _BUILD_CACHE[key] = _build(allowed)
    nc = _BUILD_CACHE[key]

    kw = {}
    if trace:
        kw["trace"] = True
        if trace_kwargs:
            kw.update(trace_kwargs)
    res = run_bass_kernel_spmd(nc, in_maps, list(range(N_CORES)), **kw)
    full = np.empty((5, B, H), np.float32)
    for c in range(N_CORES):
        full[:, c * BC : (c + 1) * BC, :] = res.results[c]["out"]
    return full, res


def kernel(**inputs):
    full, _ = _run(inputs)
    return full
